# revision 10
# baseline (speedup 1.0000x reference)
"""Group-limited MoE router kernel for Trainium2 (Bass/Tile), 8-core SPMD.

Per token (row of 256 experts):
  scores = sigmoid(logits); biased = scores + bias
  group_score[g] = top2sum(biased[g*32:(g+1)*32]) for 8 groups of 32
  keep top-4 groups, add -1e30 to the rest
  topk_ids = top-8 of masked biased (descending)
  weights  = scores[topk_ids]; renormalize to sum 1; * 2.5

Data-parallel over tokens: 131072 tokens -> 8 cores x 16384.
Tokens on SBUF partitions (128/slab), experts on the free dim.

Weight extraction uses a rank-key trick instead of a score gather:
  local_scatter rank constants {16,14,..,2} to the winning expert
  positions, add scores, MAX8 -> scores in biased-rank order, then
  subtract the rank constants.  Avoids 4 extra 256-wide passes.

Work is spread across Act (sigmoid), DVE (match ops) and GpSimd
(elementwise/reduce) so no single engine serializes the slab loop.
"""

import numpy as np

TOKENS = 131072
E = 256
G = 8
EPG = 32
K = 8
SCALE = 2.5
N_CORES = 8
TPC = TOKENS // N_CORES

P = 128
NB = 4   # slabs per small-op batch
OB = 8   # slabs per output DMA

NEG = -1.0e30
RANKV = [16.0, 14.0, 12.0, 10.0, 8.0, 6.0, 4.0, 2.0]       # bf16-exact rank keys
RANKV25 = [v * SCALE for v in RANKV]                        # 40,35,..,5 (f32-exact)


def build_kernel(tpc: int):
    import concourse.bass as bass
    import concourse.bacc as bacc
    import concourse.mybir as mybir
    from concourse.tile import TileContext

    f32 = mybir.dt.float32
    bf16 = mybir.dt.bfloat16
    u16 = mybir.dt.uint16
    i16 = mybir.dt.int16
    Alu = mybir.AluOpType
    Sigmoid = mybir.ActivationFunctionType.Sigmoid

    nc = bacc.Bacc()
    logits_d = nc.declare_dram_parameter("logits", [tpc, E], f32, isOutput=False)
    bias_d = nc.declare_dram_parameter("bias", [1, E], f32, isOutput=False)
    w_d = nc.declare_dram_parameter("weights", [tpc, K], f32, isOutput=True)
    i_d = nc.declare_dram_parameter("ids", [tpc, K], u16, isOutput=True)

    n_slab = tpc // P
    assert n_slab % OB == 0 and OB % NB == 0

    with TileContext(nc) as tc:
        with (
            tc.tile_pool(name="const", bufs=1) as cpool,
            tc.tile_pool(name="xin", bufs=3) as xpool,
            tc.tile_pool(name="scores", bufs=10) as spool,
            tc.tile_pool(name="biased", bufs=10) as bpool,
            tc.tile_pool(name="rep", bufs=4) as rpool,
            tc.tile_pool(name="masked", bufs=4) as mpool,
            tc.tile_pool(name="rgrid", bufs=4) as gpool,
            tc.tile_pool(name="key2", bufs=4) as kpool,
            tc.tile_pool(name="smallb", bufs=3) as nbpool,
            tc.tile_pool(name="tiny", bufs=6) as tpool,
            tc.tile_pool(name="out", bufs=3) as opool,
        ):
            bias_sb = cpool.tile([P, E], f32)
            nc.gpsimd.dma_start(out=bias_sb, in_=bias_d[:].to_broadcast([P, E]))
            rank16 = cpool.tile([P, K], bf16)
            rank25 = cpool.tile([P, K], f32)
            for k in range(K):
                nc.vector.memset(rank16[:, k : k + 1], RANKV[k])
                nc.vector.memset(rank25[:, k : k + 1], RANKV25[k])

            for b in range(n_slab // NB):
                # ---- batched input DMA: NB slabs in one transfer ----
                t0 = b * NB * P
                x4 = xpool.tile([P, NB, E], f32, tag="x4")
                nc.sync.dma_start(
                    out=x4,
                    in_=logits_d[t0 : t0 + NB * P, :].rearrange(
                        "(nb p) e -> p nb e", p=P
                    ),
                )

                m1b = nbpool.tile([P, NB, G], f32, tag="m1b")
                m2b = nbpool.tile([P, NB, G], f32, tag="m2b")
                scores_l = []
                biased_l = []
                for j in range(NB):
                    scores = spool.tile([P, E], f32, tag="scores")
                    nc.scalar.activation(out=scores, in_=x4[:, j, :], func=Sigmoid)
                    scores_l.append(scores)

                    biased = bpool.tile([P, E], f32, tag="biased")
                    nc.gpsimd.tensor_tensor(
                        out=biased, in0=scores, in1=bias_sb, op=Alu.add
                    )
                    biased_l.append(biased)

                    bg = biased.rearrange("p (g e) -> p g e", g=G)
                    nc.vector.tensor_reduce(
                        out=m1b[:, j, :], in_=bg, axis=mybir.AxisListType.X, op=Alu.max
                    )
                    rep = rpool.tile([P, E], f32, tag="rep")
                    nc.vector.match_replace(
                        out=rep,
                        in_to_replace=m1b[:, j, :],
                        in_values=biased,
                        imm_value=NEG,
                    )
                    nc.vector.tensor_reduce(
                        out=m2b[:, j, :],
                        in_=rep.rearrange("p (g e) -> p g e", g=G),
                        axis=mybir.AxisListType.X,
                        op=Alu.max,
                    )

                # ---- group selection (batched over NB slabs) ----
                gsb = nbpool.tile([P, NB, G], f32, tag="gsb")
                nc.gpsimd.tensor_tensor(out=gsb, in0=m1b, in1=m2b, op=Alu.add)
                g8b = nbpool.tile([P, NB, K], f32, tag="g8b")
                for j in range(NB):
                    nc.vector.max(out=g8b[:, j, :], in_=gsb[:, j, :])
                # neg = -1e30 where group score below the 4th-largest (drop group)
                ltb = nbpool.tile([P, NB, G], f32, tag="ltb")
                nc.vector.tensor_tensor(
                    out=ltb,
                    in0=gsb,
                    in1=g8b[:, :, 3:4].to_broadcast([P, NB, G]),
                    op=Alu.is_lt,
                )
                negb = nbpool.tile([P, NB, G], f32, tag="negb")
                nc.vector.tensor_scalar(
                    out=negb, in0=ltb, scalar1=NEG, scalar2=None, op0=Alu.mult
                )

                s8rb = nbpool.tile([P, NB, K], f32, tag="s8rb")
                if b % (OB // NB) == 0:
                    ids_ob = opool.tile([P, OB, K], u16, tag="ids_ob", name="ids_ob")
                    w_ob = opool.tile([P, OB, K], f32, tag="w_ob", name="w_ob")
                oj0 = (b % (OB // NB)) * NB

                for j in range(NB):
                    biased = biased_l[j]
                    masked = mpool.tile([P, E], f32, tag="masked")
                    nc.gpsimd.tensor_tensor(
                        out=masked.rearrange("p (g e) -> p g e", g=G),
                        in0=biased.rearrange("p (g e) -> p g e", g=G),
                        in1=negb[:, j, :].unsqueeze(2).to_broadcast([P, G, EPG]),
                        op=Alu.add,
                    )
                    vals8 = tpool.tile([P, K], f32, tag="vals8")
                    nc.vector.max(out=vals8, in_=masked)
                    idx8 = ids_ob[:, oj0 + j, :]
                    nc.vector.max_index(out=idx8, in_max=vals8, in_values=masked)

                    # rank-key: scatter rank consts to winner positions,
                    # add scores, MAX8 recovers scores in rank order.
                    rgrid = gpool.tile([P, E], bf16, tag="rgrid")
                    nc.gpsimd.local_scatter(
                        out_ap=rgrid,
                        data_ap=rank16,
                        idxs_ap=idx8.bitcast(i16),
                        channels=P,
                        num_elems=E,
                        num_idxs=K,
                    )
                    key2 = kpool.tile([P, E], f32, tag="key2")
                    nc.gpsimd.tensor_tensor(
                        out=key2, in0=scores_l[j], in1=rgrid, op=Alu.add
                    )
                    nc.vector.max(out=s8rb[:, j, :], in_=key2)

                # ---- weights + renorm (batched over NB slabs) ----
                w25b = nbpool.tile([P, NB, K], f32, tag="w25b")
                nc.vector.scalar_tensor_tensor(
                    out=w25b,
                    in0=s8rb,
                    scalar=SCALE,
                    in1=rank25.unsqueeze(1).to_broadcast([P, NB, K]),
                    op0=Alu.mult,
                    op1=Alu.subtract,
                )
                sum25b = tpool.tile([P, NB], f32, tag="sum25b")
                nc.vector.tensor_reduce(
                    out=sum25b, in_=w25b, axis=mybir.AxisListType.X, op=Alu.add
                )
                denomb = tpool.tile([P, NB], f32, tag="denomb")
                nc.vector.tensor_scalar(
                    out=denomb,
                    in0=sum25b,
                    scalar1=1.0 / SCALE,
                    scalar2=1.0e-20,
                    op0=Alu.mult,
                    op1=Alu.add,
                )
                rcpb = tpool.tile([P, NB], f32, tag="rcpb")
                nc.vector.reciprocal(out=rcpb, in_=denomb)
                nc.gpsimd.tensor_tensor(
                    out=w_ob[:, oj0 : oj0 + NB, :],
                    in0=w25b,
                    in1=rcpb.unsqueeze(2).to_broadcast([P, NB, K]),
                    op=Alu.mult,
                )

                # ---- batched output DMA every OB slabs ----
                if (b + 1) % (OB // NB) == 0:
                    to0 = (b + 1) * NB * P - OB * P
                    nc.sync.dma_start(
                        out=w_d[to0 : to0 + OB * P, :].rearrange(
                            "(ob p) k -> p ob k", p=P
                        ),
                        in_=w_ob,
                    )
                    nc.sync.dma_start(
                        out=i_d[to0 : to0 + OB * P, :].rearrange(
                            "(ob p) k -> p ob k", p=P
                        ),
                        in_=ids_ob,
                    )

    nc.finalize()
    return nc


_NC_CACHE = {}


def _get_nc(tpc: int):
    if tpc not in _NC_CACHE:
        _NC_CACHE[tpc] = build_kernel(tpc)
    return _NC_CACHE[tpc]


def kernel(router_logits: np.ndarray, expert_bias: np.ndarray, _trace: bool = False):
    from concourse.bass_utils import run_bass_kernel_spmd

    router_logits = np.asarray(router_logits, dtype=np.float32)
    expert_bias = np.asarray(expert_bias, dtype=np.float32)
    tokens = router_logits.shape[0]
    assert tokens % N_CORES == 0
    tpc = tokens // N_CORES

    nc = _get_nc(tpc)
    bias_in = expert_bias.reshape(1, E)
    in_maps = [
        {
            "logits": np.ascontiguousarray(router_logits[c * tpc : (c + 1) * tpc]),
            "bias": bias_in,
        }
        for c in range(N_CORES)
    ]
    res = run_bass_kernel_spmd(
        nc, in_maps, core_ids=list(range(N_CORES)), trace=_trace
    )
    weights = np.concatenate([r["weights"] for r in res.results], axis=0)
    ids = np.concatenate([r["ids"] for r in res.results], axis=0).astype(np.int32)
    if _trace:
        kernel.last_exec_time_ns = res.exec_time_ns
        kernel.last_mean_exec_time_ns = res.mean_exec_time_ns
        it = res.instructions_and_trace
        kernel.last_trace_path = it[1] if it else None
    return weights, ids


# revision 11
# speedup vs baseline: 1.8800x; 1.8800x over previous
"""Group-limited MoE router kernel for Trainium2 (Bass/Tile), 8-core SPMD.

Per token (row of 256 experts):
  scores = sigmoid(logits); biased = scores + bias
  group_score[g] = top2sum(biased[g*32:(g+1)*32]) for 8 groups of 32
  keep top-4 groups, add -1e30 to the rest
  topk_ids = top-8 of masked biased (descending)
  weights  = scores[topk_ids]; renormalize to sum 1; * 2.5

Data-parallel over tokens: 131072 tokens -> 8 cores x 16384.
Tokens on SBUF partitions (128/slab), experts on the free dim.

Weight extraction uses a rank-key trick instead of a score gather:
  local_scatter rank constants {16,14,..,2} (bf16) to the winning expert
  positions, add scores, MAX8 -> scores in biased-rank order, then
  subtract the rank constants.

Engines: Act does sigmoid + the final scale; DVE does reduces/match ops
and small batched arithmetic; GpSimd(Pool) does the three big
elementwise adds (bias, group mask, rank key) + local_scatter.

The emission order is software-pipelined: per outer iteration i, each
stage works on a different batch (i, i-1, ..., i-8) so every
cross-engine dependency is at least one full batch old and the
in-order engine queues never stall on each other.
"""

import numpy as np

TOKENS = 131072
E = 256
G = 8
EPG = 32
K = 8
SCALE = 2.5
N_CORES = 8
TPC = TOKENS // N_CORES

P = 128
NB = 4   # slabs per batch (one batch = one pipeline unit)
OB = 8   # slabs per output DMA (2 batches)

NEG = -1.0e30
RANKV = [16.0, 14.0, 12.0, 10.0, 8.0, 6.0, 4.0, 2.0]  # bf16-exact rank keys


def build_kernel(tpc: int):
    import concourse.bacc as bacc
    import concourse.mybir as mybir
    from concourse.tile import TileContext

    f32 = mybir.dt.float32
    bf16 = mybir.dt.bfloat16
    u16 = mybir.dt.uint16
    i16 = mybir.dt.int16
    Alu = mybir.AluOpType
    Sigmoid = mybir.ActivationFunctionType.Sigmoid

    nc = bacc.Bacc()
    logits_d = nc.declare_dram_parameter("logits", [tpc, E], f32, isOutput=False)
    bias_d = nc.declare_dram_parameter("bias", [1, E], f32, isOutput=False)
    w_d = nc.declare_dram_parameter("weights", [tpc, K], f32, isOutput=True)
    i_d = nc.declare_dram_parameter("ids", [tpc, K], u16, isOutput=True)

    n_slab = tpc // P
    assert n_slab % OB == 0 and OB % NB == 0
    B = n_slab // NB  # number of batches

    with TileContext(nc) as tc:
        with (
            tc.tile_pool(name="const", bufs=1) as cpool,
            tc.tile_pool(name="xin", bufs=3) as xpool,
            tc.tile_pool(name="scores", bufs=7) as spool,
            tc.tile_pool(name="biased", bufs=4) as bpool,
            tc.tile_pool(name="rep", bufs=2) as rpool,
            tc.tile_pool(name="masked", bufs=3) as mpool,
            tc.tile_pool(name="rgrid", bufs=3) as gpool,
            tc.tile_pool(name="key2", bufs=3) as kpool,
            tc.tile_pool(name="smallb", bufs=8) as nbpool,
            tc.tile_pool(name="tiny", bufs=8) as tpool,
            tc.tile_pool(name="out", bufs=3) as opool,
        ):
            bias_sb = cpool.tile([P, E], f32)
            nc.gpsimd.dma_start(out=bias_sb, in_=bias_d[:].to_broadcast([P, E]))
            rank16 = cpool.tile([P, K], bf16)
            rank32 = cpool.tile([P, K], f32)
            for k in range(K):
                nc.vector.memset(rank16[:, k : k + 1], RANKV[k])
                nc.vector.memset(rank32[:, k : k + 1], RANKV[k])

            # per-batch state carried between pipeline stages
            st = {}

            def stage_in(b):
                t0 = b * NB * P
                x = xpool.tile([P, NB, E], f32, tag="x", name="x")
                nc.sync.dma_start(
                    out=x,
                    in_=logits_d[t0 : t0 + NB * P, :].rearrange(
                        "(nb p) e -> p nb e", p=P
                    ),
                )
                st[b] = {"x": x}

            def stage_sigmoid(b):
                s = st[b]
                scores = spool.tile([P, NB, E], f32, tag="scores", name="scores")
                for j in range(NB):
                    nc.scalar.activation(
                        out=scores[:, j, :], in_=s["x"][:, j, :], func=Sigmoid
                    )
                s["scores"] = scores
                s["x"] = None

            def stage_biased(b):
                s = st[b]
                biased = bpool.tile([P, NB, E], f32, tag="biased", name="biased")
                nc.gpsimd.tensor_tensor(
                    out=biased,
                    in0=s["scores"],
                    in1=bias_sb.unsqueeze(1).to_broadcast([P, NB, E]),
                    op=Alu.add,
                )
                s["biased"] = biased

            def stage_groups(b):
                s = st[b]
                biased = s["biased"]
                bg = biased.rearrange("p nb (g e) -> p nb g e", g=G)
                m1b = nbpool.tile([P, NB, G], f32, tag="m1b", name="m1b")
                nc.vector.tensor_reduce(
                    out=m1b, in_=bg, axis=mybir.AxisListType.X, op=Alu.max
                )
                rep = rpool.tile([P, NB, E], f32, tag="rep", name="rep")
                for j in range(NB):
                    nc.vector.match_replace(
                        out=rep[:, j, :],
                        in_to_replace=m1b[:, j, :],
                        in_values=biased[:, j, :],
                        imm_value=NEG,
                    )
                m2b = nbpool.tile([P, NB, G], f32, tag="m2b", name="m2b")
                nc.vector.tensor_reduce(
                    out=m2b,
                    in_=rep.rearrange("p nb (g e) -> p nb g e", g=G),
                    axis=mybir.AxisListType.X,
                    op=Alu.max,
                )
                gsb = nbpool.tile([P, NB, G], f32, tag="gsb", name="gsb")
                nc.vector.tensor_tensor(out=gsb, in0=m1b, in1=m2b, op=Alu.add)
                g8b = nbpool.tile([P, NB, K], f32, tag="g8b", name="g8b")
                for j in range(NB):
                    nc.vector.max(out=g8b[:, j, :], in_=gsb[:, j, :])
                # neg = NEG where group score below the 4th-largest (drop group)
                ltb = nbpool.tile([P, NB, G], f32, tag="ltb", name="ltb")
                nc.vector.tensor_tensor(
                    out=ltb,
                    in0=gsb,
                    in1=g8b[:, :, 3:4].to_broadcast([P, NB, G]),
                    op=Alu.is_lt,
                )
                negb = nbpool.tile([P, NB, G], f32, tag="negb", name="negb")
                nc.vector.tensor_scalar(
                    out=negb, in0=ltb, scalar1=NEG, scalar2=None, op0=Alu.mult
                )
                s["negb"] = negb

            def stage_masked(b):
                s = st[b]
                masked = mpool.tile([P, NB, E], f32, tag="masked", name="masked")
                nc.gpsimd.tensor_tensor(
                    out=masked.rearrange("p nb (g e) -> p nb g e", g=G),
                    in0=s["biased"].rearrange("p nb (g e) -> p nb g e", g=G),
                    in1=s["negb"].unsqueeze(3).to_broadcast([P, NB, G, EPG]),
                    op=Alu.add,
                )
                s["masked"] = masked
                s["biased"] = None

            def stage_search(b):
                s = st[b]
                masked = s["masked"]
                if b % (OB // NB) == 0:
                    s["ids_ob"] = opool.tile(
                        [P, OB, K], u16, tag="ids_ob", name="ids_ob"
                    )
                    s["w_ob"] = opool.tile([P, OB, K], f32, tag="w_ob", name="w_ob")
                else:
                    s["ids_ob"] = st[b - 1]["ids_ob"]
                    s["w_ob"] = st[b - 1]["w_ob"]
                oj0 = (b % (OB // NB)) * NB
                vals8b = nbpool.tile([P, NB, K], f32, tag="vals8b", name="vals8b")
                for j in range(NB):
                    nc.vector.max(out=vals8b[:, j, :], in_=masked[:, j, :])
                    nc.vector.max_index(
                        out=s["ids_ob"][:, oj0 + j, :],
                        in_max=vals8b[:, j, :],
                        in_values=masked[:, j, :],
                    )
                s["oj0"] = oj0
                s["masked"] = None

            def stage_rankkey(b):
                s = st[b]
                rgrid = gpool.tile([P, NB, E], bf16, tag="rgrid", name="rgrid")
                for j in range(NB):
                    nc.gpsimd.local_scatter(
                        out_ap=rgrid[:, j, :],
                        data_ap=rank16,
                        idxs_ap=s["ids_ob"][:, s["oj0"] + j, :].bitcast(i16),
                        channels=P,
                        num_elems=E,
                        num_idxs=K,
                    )
                key2 = kpool.tile([P, NB, E], f32, tag="key2", name="key2")
                nc.gpsimd.tensor_tensor(
                    out=key2, in0=s["scores"], in1=rgrid, op=Alu.add
                )
                s["key2"] = key2
                s["scores"] = None

            def stage_final(b):
                s = st[b]
                key2 = s["key2"]
                s8rb = nbpool.tile([P, NB, K], f32, tag="s8rb", name="s8rb")
                for j in range(NB):
                    nc.vector.max(out=s8rb[:, j, :], in_=key2[:, j, :])
                # w = s8r - rank ; renorm: wout = w * (2.5 / (sum w + 1e-20))
                w8b = nbpool.tile([P, NB, K], f32, tag="w8b", name="w8b")
                nc.vector.tensor_tensor(
                    out=w8b,
                    in0=s8rb,
                    in1=rank32.unsqueeze(1).to_broadcast([P, NB, K]),
                    op=Alu.subtract,
                )
                sumb = tpool.tile([P, NB], f32, tag="sumb", name="sumb")
                nc.vector.tensor_reduce(
                    out=sumb, in_=w8b, axis=mybir.AxisListType.X, op=Alu.add
                )
                nc.vector.tensor_scalar(
                    out=sumb, in0=sumb, scalar1=1.0e-20, scalar2=None, op0=Alu.add
                )
                rcpb = tpool.tile([P, NB], f32, tag="rcpb", name="rcpb")
                nc.vector.reciprocal(out=rcpb, in_=sumb)
                nc.vector.tensor_scalar(
                    out=rcpb, in0=rcpb, scalar1=SCALE, scalar2=None, op0=Alu.mult
                )
                for j in range(NB):
                    nc.scalar.mul(
                        out=s["w_ob"][:, s["oj0"] + j, :],
                        in_=w8b[:, j, :],
                        mul=rcpb[:, j : j + 1],
                    )
                s["key2"] = None

            def stage_out(b):
                # one output DMA pair per OB slabs (after the last batch of
                # the OB window finished stage_final)
                if (b + 1) % (OB // NB) != 0:
                    return
                s = st[b]
                to0 = (b + 1) * NB * P - OB * P
                nc.sync.dma_start(
                    out=w_d[to0 : to0 + OB * P, :].rearrange(
                        "(ob p) k -> p ob k", p=P
                    ),
                    in_=s["w_ob"],
                )
                nc.sync.dma_start(
                    out=i_d[to0 : to0 + OB * P, :].rearrange(
                        "(ob p) k -> p ob k", p=P
                    ),
                    in_=s["ids_ob"],
                )

            stages = [
                stage_in,        # works on batch i
                stage_sigmoid,   # i-1
                stage_biased,    # i-2
                stage_groups,    # i-3
                stage_masked,    # i-4
                stage_search,    # i-5
                stage_rankkey,   # i-6
                stage_final,     # i-7
                stage_out,       # i-8
            ]
            D = len(stages)
            for i in range(B + D - 1):
                for k, stage in enumerate(stages):
                    b = i - k
                    if 0 <= b < B:
                        stage(b)
                drop = i - D
                if drop >= 0 and drop in st:
                    del st[drop]

    nc.finalize()
    return nc


_NC_CACHE = {}


def _get_nc(tpc: int):
    if tpc not in _NC_CACHE:
        _NC_CACHE[tpc] = build_kernel(tpc)
    return _NC_CACHE[tpc]


def kernel(router_logits: np.ndarray, expert_bias: np.ndarray, _trace: bool = False):
    from concourse.bass_utils import run_bass_kernel_spmd

    router_logits = np.asarray(router_logits, dtype=np.float32)
    expert_bias = np.asarray(expert_bias, dtype=np.float32)
    tokens = router_logits.shape[0]
    assert tokens % N_CORES == 0
    tpc = tokens // N_CORES

    nc = _get_nc(tpc)
    bias_in = expert_bias.reshape(1, E)
    in_maps = [
        {
            "logits": np.ascontiguousarray(router_logits[c * tpc : (c + 1) * tpc]),
            "bias": bias_in,
        }
        for c in range(N_CORES)
    ]
    res = run_bass_kernel_spmd(
        nc, in_maps, core_ids=list(range(N_CORES)), trace=_trace
    )
    weights = np.concatenate([r["weights"] for r in res.results], axis=0)
    ids = np.concatenate([r["ids"] for r in res.results], axis=0).astype(np.int32)
    if _trace:
        kernel.last_exec_time_ns = res.exec_time_ns
        kernel.last_mean_exec_time_ns = res.mean_exec_time_ns
        it = res.instructions_and_trace
        kernel.last_trace_path = it[1] if it else None
    return weights, ids
